# revision 27
# baseline (speedup 1.0000x reference)
"""Soft-MoE layer (B=1024, I=512, O=512, E=16) on 8 TRN2 NeuronCores.

Strategy: output-column sharding (no collectives). Core c owns output
columns [64c : 64c+64] and computes, for the full batch and ALL 16 experts,
    out[b, oc] = sum_e coeffs[b, e] * (x[b] @ W[e][:, oc] + bias[e][oc])
The host concatenates the 8 column slices. ncfw collectives measured
~100+us for a 2MB 8-rank ReduceScatter (latency-floor dominated), so the
expert reduction is done locally on DVE instead: PE computes per-expert
partials for the core's 64 columns, packed 8-experts-per-matmul along the
free dim (N=512, full PE efficiency), then DVE applies the per-sample
coefficients (stride-0 broadcast APs) and reduces over experts.

Details:
  - x is staged host-side transposed (xT) so the contraction dim I lands on
    SBUF partitions; matmuls run in float32r (full-rate fp32 streaming, 12
    mantissa bits — host pre-rounds operands to match).
  - bias[e] is folded into the PSUM accumulation with one extra matmul per
    psum tile: lhsT = const 1/128, rhs = host-broadcast biases. The PSUM
    tile then holds exactly (x@W[e] + bias[e]) per expert block, so the
    coefficient weighting afterwards is correct.
"""

import numpy as np

import concourse.bass as bass
import concourse.bacc as bacc
import concourse.mybir as mybir
import concourse.tile as tile
from concourse.bass_utils import run_bass_kernel_spmd

B, I, O, E = 1024, 512, 512, 16
NCORES = 8
OC = O // NCORES  # output columns per core = 64
BT = B // 128  # batch tiles = 8
KT = I // 128  # contraction chunks = 4
EH = E // 2  # experts per psum half = 8

F32 = mybir.dt.float32
F32R = mybir.dt.float32r
BF16 = mybir.dt.bfloat16

_cache = {}


def _build(loop_n=None):
    """loop_n: if set, wrap the per-iteration body in a hardware For_i loop
    (benchmark amplification only)."""
    nc = bacc.Bacc(
        "TRN2",
        target_bir_lowering=False,
        debug=False,
        num_devices=NCORES,
    )

    xt_d = nc.dram_tensor("xt", [128, KT, B], F32R, kind="ExternalInput")
    w_d = nc.dram_tensor("w", [128, KT, E, OC], F32R, kind="ExternalInput")
    ct_d = nc.dram_tensor("ct", [E, B], F32R, kind="ExternalInput")
    biasoc_d = nc.dram_tensor("biasoc", [E, OC], F32R, kind="ExternalInput")
    c2_d = nc.dram_tensor("c2", [128, BT, E], F32, kind="ExternalInput")
    out_d = nc.dram_tensor("out", [B, OC], F32, kind="ExternalOutput")

    with tile.TileContext(nc) as tc:
        with (
            tc.tile_pool(name="const", bufs=1) as const,
            tc.tile_pool(name="psum", bufs=2, space="PSUM") as psum,
            tc.tile_pool(name="stage", bufs=3) as stage,
        ):
            xt_sb = const.tile([128, KT, B], F32R, tag="xt")
            w_sb = const.tile([128, KT, E, OC], F32R, tag="w")
            # split per k-chunk so the first matmuls can start ~4x sooner
            for k in range(KT):
                nc.sync.dma_start(xt_sb[:, k, :], xt_d[:, k, :])
                nc.sync.dma_start(w_sb[:, k, :, :], w_d[:, k, :, :])
            ct_sb = const.tile([E, B], F32R, tag="ct")
            nc.sync.dma_start(ct_sb[:], ct_d[:])
            biasoc_sb = const.tile([E, OC], F32R, tag="biasoc")
            nc.sync.dma_start(biasoc_sb[:], biasoc_d[:])
            c2_sb = const.tile([128, BT, E], F32, tag="c2")
            nc.sync.dma_start(c2_sb[:], c2_d[:])

            def body():
                # all 8 batch-tiles' outputs land in one tile -> single DMA
                out_big = stage.tile([128, BT, OC], F32, tag="outbig")
                for i in range(BT):
                    bs = slice(128 * i, 128 * (i + 1))
                    # m is oc-major bf16 so the expert reduce reads packed
                    # 2-byte innermost (DVE 2x/4x mode)
                    m = stage.tile([128, OC, E], BF16, tag="m")
                    # one 2-bank psum tile holds all 16 experts' partials
                    pse = psum.tile([128, E, OC], F32, tag="ps")
                    for h in range(2):
                        es = slice(EH * h, EH * (h + 1))
                        for k in range(KT):
                            nc.tensor.matmul(
                                pse[:, es, :],
                                xt_sb[:, k, bs],
                                w_sb[:, k, es, :],
                                start=(k == 0),
                                stop=(k == KT - 1),
                            )
                    # m[:, oc, e] = pse[:, e, oc] * coeffs[b, e] (one op)
                    cb = (
                        c2_sb[:, i, :].unsqueeze(2).broadcast_to([128, E, OC])
                    )
                    nc.vector.tensor_mul(
                        m[:].transpose([0, 2, 1]), pse[:], cb
                    )
                    # bias term: psb[b, oc] = sum_e coeffs[b, e] bias[e, oc]
                    psb = psum.tile([128, OC], F32, tag="psb")
                    nc.tensor.matmul(
                        psb[:],
                        ct_sb[:, bs],
                        biasoc_sb[:],
                        start=True,
                        stop=True,
                    )
                    # tree-reduce over experts; stage 1 (biggest) runs on the
                    # otherwise-idle GpSimd, the rest in bf16 2x on DVE
                    with nc.allow_low_precision("16-term expert sum"):
                        t1 = stage.tile([128, OC, 8], BF16, tag="t1")
                        nc.gpsimd.tensor_add(
                            t1[:], m[:, :, 0:8], m[:, :, 8:16]
                        )
                        t2 = stage.tile([128, OC, 4], BF16, tag="t2")
                        nc.vector.tensor_add(
                            t2[:], t1[:, :, 0:4], t1[:, :, 4:8]
                        )
                        t3 = stage.tile([128, OC, 2], BF16, tag="t3")
                        nc.vector.tensor_add(
                            t3[:], t2[:, :, 0:2], t2[:, :, 2:4]
                        )
                        red = stage.tile([128, OC], BF16, tag="red")
                        nc.vector.tensor_add(
                            red[:], t3[:, :, 0], t3[:, :, 1]
                        )
                    nc.vector.tensor_add(out_big[:, i, :], red[:], psb[:])
                    # flush every 4 batch-tiles: amortizes DMA issue cost
                    # without serializing the whole output into the tail
                    if i % 4 == 3:
                        g = slice(i - 3, i + 1)
                        rows = slice(128 * (i - 3), 128 * (i + 1))
                        nc.sync.dma_start(
                            out_d[rows, :].rearrange(
                                "(i p) o -> p i o", p=128
                            ),
                            out_big[:, g, :],
                        )

            if loop_n is not None:
                with tc.For_i(0, loop_n, 1):
                    body()
            else:
                body()

    nc.compile()
    return nc


def _round_fp32r(a):
    """Round fp32 to fp32r (12 explicit mantissa bits, round-to-nearest) so
    host data matches what the PE datapath consumes."""
    bits = np.ascontiguousarray(a, dtype=np.float32).view(np.uint32)
    r = ((bits.astype(np.uint64) + 0x800) & ~np.uint64(0xFFF)).astype(np.uint32)
    return r.view(np.float32)


def _prep_in_maps(x, coeffs, expert_weights, expert_biases):
    x = _round_fp32r(np.ascontiguousarray(x, dtype=np.float32))
    coeffs = np.ascontiguousarray(coeffs, dtype=np.float32)
    expert_weights = _round_fp32r(
        np.ascontiguousarray(expert_weights, dtype=np.float32)
    )
    expert_biases = _round_fp32r(
        np.ascontiguousarray(expert_biases, dtype=np.float32)
    )

    # xT [I, B] -> [128, KT, B]: partition p, chunk k holds x[:, k*128+p]
    xt = np.ascontiguousarray(x.T.reshape(KT, 128, B).transpose(1, 0, 2))
    # coeffs as per-partition scalars: [128, BT, E]
    c2 = np.ascontiguousarray(coeffs.reshape(BT, 128, E).transpose(1, 0, 2))
    # coeffs transposed for the bias matmul lhsT: [E, B]
    ct = _round_fp32r(np.ascontiguousarray(coeffs.T))

    in_maps = []
    for c in range(NCORES):
        ocs = slice(OC * c, OC * (c + 1))
        # W[e][I, oc] -> [128, KT, E, OC]
        w = np.ascontiguousarray(
            expert_weights[:, :, ocs]
            .reshape(E, KT, 128, OC)
            .transpose(2, 1, 0, 3)
        )
        biasoc = np.ascontiguousarray(expert_biases[:, ocs])
        in_maps.append(
            {"xt": xt, "w": w, "ct": ct, "biasoc": biasoc, "c2": c2}
        )
    return in_maps


def _run(inputs, **kwargs):
    if "nc" not in _cache:
        _cache["nc"] = _build()
    nc = _cache["nc"]
    in_maps = _prep_in_maps(**inputs)
    res = run_bass_kernel_spmd(nc, in_maps, list(range(NCORES)), **kwargs)
    out = np.concatenate(
        [np.asarray(res.results[c]["out"]) for c in range(NCORES)], axis=1
    )
    return out.astype(np.float32), res


def kernel(**inputs):
    out, _ = _run(inputs)
    return out
